# revision 1
# baseline (speedup 1.0000x reference)
"""Multi-head attention (B=4, N=2048, C=768, H=12) on 8 trn2 NeuronCores.

Sharding: core c handles batch b = c//2 and query rows [ (c%2)*1024, +1024 ).
Each core computes K/V for its full batch (duplicated across the pair),
attention for all 12 heads over its 1024 queries, and the output projection
for its rows. Output gather is pure concatenation (no cross-core reduce).

On-chip layout (per core):
  xt  = x_b.T           [768, 2048]   (c on partitions)
  QT  = Wq @ xq.T       [768, 1024]   head h rows h*64..h*64+63
  KT  = Wk @ x.T        [768, 2048]
  V   = x @ Wv.T        [2048, 780]   row-tiles of 128 keys; head h in cols
                                      h*65..h*65+63, col h*65+64 == 1.0 (ones
                                      column -> PV matmul also emits softmax
                                      denominators)
  ST_h = (K_h Q_h^T)    [128k, 1024q] PSUM per k-tile; exp on ScalarE with
                                      the 1/8 attention scale folded in
  OT_h = [V_h|1]^T P_h  [65, 1024]    PSUM accumulated over 16 k-tiles;
                                      row 64 = softmax denominators
  Y    = OT^T Wp^T + bp [1024, 768]
"""

import os
import sys

import numpy as np

sys.path.insert(0, "/opt/trn_rl_repo")

import concourse.bass as bass
from concourse import bacc
import concourse.mybir as mybir
from concourse.tile import TileContext
from concourse.bass_utils import run_bass_kernel_spmd
from concourse.dma_utils import dma_copy

P = 128
C = 768
NK = 2048
NQ = 1024
H = 12
DH = 64
CT = C // P          # 6 c-tiles (contraction tiles for the linears)
KT = NK // P         # 16 key tiles
QCH = 512            # q-chunk (max psum bank free dim for fp32)
NQC = NQ // QCH      # 2 q chunks
SCALE = DH ** -0.5
F32 = mybir.dt.float32
BF16 = mybir.dt.bfloat16

LAST_RESULT = None
_PROG = None


def _build_program() -> bass.Bass:
    nc = bacc.Bacc(None, target_bir_lowering=False)

    xt = nc.dram_tensor("xt", [C, NK], F32, kind="ExternalInput")
    xqt = nc.dram_tensor("xqt", [C, NQ], F32, kind="ExternalInput")
    wqt = nc.dram_tensor("wqt", [C, C], F32, kind="ExternalInput")
    wkt = nc.dram_tensor("wkt", [C, C], F32, kind="ExternalInput")
    wvt = nc.dram_tensor("wvt", [C, C], F32, kind="ExternalInput")
    wpt = nc.dram_tensor("wpt", [C, C], F32, kind="ExternalInput")
    bp = nc.dram_tensor("bp", [1, C], F32, kind="ExternalInput")
    y = nc.dram_tensor("y", [NQ, C], F32, kind="ExternalOutput")

    with TileContext(nc) as tc:
        with (
            tc.tile_pool(name="persist", bufs=1) as persist,
            tc.tile_pool(name="pt", bufs=6) as ptp,
            tc.tile_pool(name="small", bufs=2) as small,
            tc.tile_pool(name="ysb", bufs=2) as ysb,
            tc.tile_pool(name="psa", bufs=2, space="PSUM") as psa,
            tc.tile_pool(name="psb", bufs=2, space="PSUM") as psb,
        ):
            # ---- load weights/activations, casting to bf16 in the DMA ----
            def load_cast(dram, rows, cols, tag):
                tiles = []
                for i in range(rows // P):
                    t = persist.tile([P, cols], BF16, tag=f"{tag}{i}")
                    dma_copy(nc.gpsimd, t[:, :], dram[i * P:(i + 1) * P, :])
                    tiles.append(t)
                return tiles

            xtb = load_cast(xt, C, NK, "xtb")
            xqb = load_cast(xqt, C, NQ, "xqb")
            wqb = load_cast(wqt, C, C, "wqb")
            wkb = load_cast(wkt, C, C, "wkb")
            wvb = load_cast(wvt, C, C, "wvb")
            wpb = load_cast(wpt, C, C, "wpb")

            bpb = persist.tile([1, C], BF16, tag="bpb")
            dma_copy(nc.gpsimd, bpb[:, :], bp[:, :])

            ones = persist.tile([P, P], BF16, tag="ones")
            nc.gpsimd.memset(ones[:, :], 1.0)
            onesf = persist.tile([DH, DH], F32, tag="onesf")
            nc.gpsimd.memset(onesf[:, :], 1.0)

            # ---- QT / KT: W @ x.T   [C, n] ----
            def wx(wtiles, xtiles, n, tag):
                out_tiles = []
                for i in range(CT):
                    t = persist.tile([P, n], BF16, tag=f"{tag}{i}")
                    out_tiles.append(t)
                for i in range(CT):          # output row tile (cout)
                    for j in range(n // QCH):  # n chunk
                        ps = psa.tile([P, QCH], F32, tag="a")
                        for k in range(CT):  # contraction c tile
                            nc.tensor.matmul(
                                ps[:, :],
                                lhsT=wtiles[k][:, i * P:(i + 1) * P],
                                rhs=xtiles[k][:, j * QCH:(j + 1) * QCH],
                                start=(k == 0),
                                stop=(k == CT - 1),
                            )
                        nc.vector.tensor_copy(
                            out_tiles[i][:, j * QCH:(j + 1) * QCH], ps[:, :]
                        )
                return out_tiles

            qtb = wx(wqb, xqb, NQ, "qt")
            ktb = wx(wkb, xtb, NK, "kt")

            # ---- V = x @ Wv.T  [2048, 12*65], ones column per head ----
            vtb = []
            for i in range(KT):
                t = persist.tile([P, H * (DH + 1)], BF16, tag=f"v{i}")
                nc.gpsimd.memset(t[:, :], 1.0)
                vtb.append(t)
            for i in range(KT):              # key row tile
                for (c0, csz) in ((0, QCH), (QCH, C - QCH)):
                    ps = psb.tile([P, csz], F32, tag="b")
                    for k in range(CT):
                        nc.tensor.matmul(
                            ps[:, :],
                            lhsT=xtb[k][:, i * P:(i + 1) * P],
                            rhs=wvb[k][:, c0:c0 + csz],
                            start=(k == 0),
                            stop=(k == CT - 1),
                        )
                    # scatter heads into stride-65 columns (leaves ones col)
                    nh = csz // DH
                    h0 = c0 // DH
                    dst = vtb[i][:, :].rearrange(
                        "p (h e) -> p h e", e=DH + 1
                    )[:, h0:h0 + nh, 0:DH]
                    src = ps[:, :].rearrange("p (h e) -> p h e", e=DH)
                    nc.vector.tensor_copy(dst, src)

            # ---- attention, head pairs ----
            otb = []   # [hd, q] bf16, tile hp holds heads 2hp (0:64), 2hp+1
            for hp in range(CT):
                t = persist.tile([P, NQ], BF16, tag=f"ot{hp}")
                otb.append(t)

            def norm_dve_front(hp, osb, den):
                # one reciprocal for both heads' denominators, which live at
                # partitions 0 and 32 (engine APs need 32-aligned bases)
                rec = small.tile([DH, NQ], F32, tag="rec")
                nc.vector.reciprocal(rec[0:1, :], den[0:1, :])
                nc.vector.reciprocal(rec[32:33, :], den[32:33, :])
                recb = small.tile([DH, NQ], BF16, tag="recb")
                nc.vector.tensor_copy(recb[0:1, :], rec[0:1, :])
                nc.vector.tensor_copy(recb[32:33, :], rec[32:33, :])
                return recb

            def norm_tail(hp, osb, recs):
                # PE broadcast of 1/denom + fused normalize-multiply
                for idx, r in ((0, recs[0]), (1, recs[1])):
                    ob = 32 * idx
                    rb = psa.tile([DH, NQ], F32, tag="a")
                    for j in range(NQC):
                        nc.tensor.matmul(
                            rb[:, j * QCH:(j + 1) * QCH],
                            lhsT=ones[ob:ob + 1, 0:DH],
                            rhs=r[0:1, j * QCH:(j + 1) * QCH],
                            start=True, stop=True,
                        )
                    nc.vector.tensor_mul(
                        otb[hp][idx * DH:(idx + 1) * DH, :],
                        osb[idx * DH:(idx + 1) * DH, :],
                        rb[0:DH, :],
                    )

            prev = None   # (hp, osb) awaiting normalization
            for hp in range(CT):
                h0, h1 = 2 * hp, 2 * hp + 1
                if prev is not None:
                    rec = norm_dve_front(*prev)
                    prev_recs = (rec[0:1, :], rec[32:33, :])
                ot0 = psb.tile([DH + 1, NQ], F32, tag="b")
                ot1 = psb.tile([DH + 1, NQ], F32, tag="b")
                def av_pair(i, pt0, pt1):
                    for j in range(NQC):
                        nc.tensor.matmul(
                            ot0[:, j * QCH:(j + 1) * QCH],
                            lhsT=vtb[i][:, h0 * (DH + 1):h0 * (DH + 1) + DH + 1],
                            rhs=pt0[:, j * QCH:(j + 1) * QCH],
                            start=(i == 0), stop=(i == KT - 1),
                        )
                        nc.tensor.matmul(
                            ot1[:, j * QCH:(j + 1) * QCH],
                            lhsT=vtb[i][:, h1 * (DH + 1):h1 * (DH + 1) + DH + 1],
                            rhs=pt1[:, j * QCH:(j + 1) * QCH],
                            start=(i == 0), stop=(i == KT - 1),
                        )

                # software pipeline: AV(i-1) is emitted between ST(i) and
                # exp(i), so the PE never stalls on the exp it just fed
                pending = None
                for i in range(KT):
                    st0 = psa.tile([P, NQ], F32, tag="a")
                    st1 = psa.tile([P, NQ], F32, tag="a")
                    for j in range(NQC):
                        # heads alternate PE row groups (base 0 / base 64)
                        # -> hardware runs the pair concurrently
                        nc.tensor.matmul(
                            st0[:, j * QCH:(j + 1) * QCH],
                            lhsT=ktb[hp][0:DH, i * P:(i + 1) * P],
                            rhs=qtb[hp][0:DH, j * QCH:(j + 1) * QCH],
                            start=True, stop=True,
                            tile_position=(0, 0),
                        )
                        nc.tensor.matmul(
                            st1[:, j * QCH:(j + 1) * QCH],
                            lhsT=ktb[hp][DH:P, i * P:(i + 1) * P],
                            rhs=qtb[hp][DH:P, j * QCH:(j + 1) * QCH],
                            start=True, stop=True,
                            tile_position=(64, 0),
                        )
                    if pending is not None:
                        av_pair(*pending)
                    pt0 = ptp.tile([P, NQ], BF16, tag="pt")
                    pt1 = ptp.tile([P, NQ], BF16, tag="pt")
                    nc.scalar.activation(
                        pt0[:, :], st0[:, :],
                        mybir.ActivationFunctionType.Exp, scale=SCALE,
                    )
                    nc.scalar.activation(
                        pt1[:, :], st1[:, :],
                        mybir.ActivationFunctionType.Exp, scale=SCALE,
                    )
                    pending = (i, pt0, pt1)
                av_pair(*pending)
                if prev is not None:
                    norm_tail(prev[0], prev[1], prev_recs)
                # drain this pair's PSUM fast so the next pair can start
                osb = small.tile([P, NQ], F32, tag="osb")
                den = small.tile([DH, NQ], F32, tag="den")
                nc.vector.tensor_copy(osb[0:DH, :], ot0[0:DH, :])
                nc.vector.tensor_copy(den[0:1, :], ot0[DH:DH + 1, :])
                nc.vector.tensor_copy(osb[DH:P, :], ot1[0:DH, :])
                nc.vector.tensor_copy(den[32:33, :], ot1[DH:DH + 1, :])
                prev = (hp, osb, den)

            # flush the last pair
            rec = norm_dve_front(*prev)
            norm_tail(prev[0], prev[1], (rec[0:1, :], rec[32:33, :]))

            # ---- projection: Y[q, co] = OT.T @ WpT + bp ----
            for qi in range(NQ // P):
                yt = ysb.tile([P, C], F32, tag="y")
                for (c0, csz) in ((0, QCH), (QCH, C - QCH)):
                    ps = psa.tile([P, csz], F32, tag="a")
                    for k in range(CT):
                        nc.tensor.matmul(
                            ps[:, :],
                            lhsT=otb[k][:, qi * P:(qi + 1) * P],
                            rhs=wpb[k][:, c0:c0 + csz],
                            start=(k == 0), stop=False,
                        )
                    nc.tensor.matmul(
                        ps[:, :],
                        lhsT=ones[0:1, 0:P],
                        rhs=bpb[0:1, c0:c0 + csz],
                        start=False, stop=True,
                    )
                    nc.vector.tensor_copy(yt[:, c0:c0 + csz], ps[:, :])
                nc.sync.dma_start(out=y[qi * P:(qi + 1) * P, :], in_=yt[:, :])

    nc.compile()
    return nc


def _get_prog() -> bass.Bass:
    global _PROG
    if _PROG is None:
        _PROG = _build_program()
    return _PROG


def kernel(x, Wq, Wk, Wv, Wp, bp):
    global LAST_RESULT
    x = np.asarray(x, dtype=np.float32)
    wqt = np.ascontiguousarray(np.asarray(Wq, np.float32).T)
    wkt = np.ascontiguousarray(np.asarray(Wk, np.float32).T)
    wvt = np.ascontiguousarray(np.asarray(Wv, np.float32).T)
    wpt = np.ascontiguousarray(np.asarray(Wp, np.float32).T)
    bpv = np.ascontiguousarray(np.asarray(bp, np.float32).reshape(1, C))

    B, N, _ = x.shape
    in_maps = []
    for core in range(8):
        b, qh = core // 2, core % 2
        xt = np.ascontiguousarray(x[b].T)
        xqt = np.ascontiguousarray(xt[:, qh * NQ:(qh + 1) * NQ])
        in_maps.append({
            "xt": xt, "xqt": xqt,
            "wqt": wqt, "wkt": wkt, "wvt": wvt, "wpt": wpt, "bp": bpv,
        })

    res = run_bass_kernel_spmd(
        _get_prog(), in_maps, core_ids=list(range(8)),
        trace=bool(os.environ.get("BASS_TRACE")),
    )
    LAST_RESULT = res

    out = np.empty((B, N, C), np.float32)
    for core in range(8):
        b, qh = core // 2, core % 2
        out[b, qh * NQ:(qh + 1) * NQ, :] = res.results[core]["y"]
    return out



# revision 3
# speedup vs baseline: 1.0125x; 1.0125x over previous
"""Multi-head attention (B=4, N=2048, C=768, H=12) on 8 trn2 NeuronCores.

Sharding: core c handles batch b = c//2 and query rows [ (c%2)*1024, +1024 ).
Each core computes K/V for its full batch (duplicated across the pair),
attention for all 12 heads over its 1024 queries, and the output projection
for its rows. Output gather is pure concatenation (no cross-core reduce).

Host side: all activations/weights are pre-cast to bf16 and x is column-
rotated per core so the core's own query columns are always x.T[:, 0:1024]
(key order is permuted identically in K and V, which leaves attention
invariant). This keeps one SPMD program for all 8 cores.

On-chip layout (per core):
  xt  = x_b.T (rolled)  [768, 2048]  bf16, c on partitions
  QT  = Wq @ xq.T       [768, 1024]  head h rows h*64..h*64+63
  KT  = Wk @ x.T        [768, 2048]
  V   = x @ Wv.T        [2048, 780]  row-tiles of 128 keys; head h in cols
                                     h*65..h*65+63, col h*65+64 == 1.0 (ones
                                     column -> PV matmul also emits softmax
                                     denominators)
  ST_h = (K_h Q_h^T)    [128k, 1024q] PSUM per k-tile; exp on ScalarE with
                                     the 1/8 attention scale folded in
  OT_h = [V_h|1]^T P_h  [65, 1024]   PSUM accumulated over 16 k-tiles;
                                     row 64 = softmax denominators
  norm: denb (bf16) <- den rows; rb = E^T @ denb broadcasts den0 to
        partitions 0..63 and den1 to 64..127 on the PE; reciprocal+multiply
        run full-width on the DVE (no single-partition ops anywhere).
  Y    = OT^T Wp^T + bp [1024, 768]  bias added by the DVE during drain

Program order interleaves the per-pair linears (QT/KT/V chunks) between
attention pairs so the PE has independent work while a pair's denominators
drain, and the Scalar engine (exp) starts as early as possible.
"""

import os
import sys

import numpy as np

sys.path.insert(0, "/opt/trn_rl_repo")

import concourse.bass as bass
from concourse import bacc
import concourse.mybir as mybir
from concourse.tile import TileContext
from concourse.bass_utils import run_bass_kernel_spmd

P = 128
C = 768
NK = 2048
NQ = 1024
H = 12
DH = 64
CT = C // P          # 6 c-tiles (contraction tiles for the linears)
KT = NK // P         # 16 key tiles
QCH = 512            # max psum bank free dim for fp32
SCALE = DH ** -0.5
F32 = mybir.dt.float32
BF16 = mybir.dt.bfloat16

LAST_RESULT = None
_PROG = None


def _build_program() -> bass.Bass:
    nc = bacc.Bacc(None, target_bir_lowering=False)

    xt = nc.dram_tensor("xt", [C, NK], BF16, kind="ExternalInput")
    wqt = nc.dram_tensor("wqt", [C, C], BF16, kind="ExternalInput")
    wkt = nc.dram_tensor("wkt", [C, C], BF16, kind="ExternalInput")
    wvt = nc.dram_tensor("wvt", [C, C], BF16, kind="ExternalInput")
    wpt = nc.dram_tensor("wpt", [C, C], BF16, kind="ExternalInput")
    bpb = nc.dram_tensor("bpb", [P, C], F32, kind="ExternalInput")
    y = nc.dram_tensor("y", [NQ, C], F32, kind="ExternalOutput")

    with TileContext(nc) as tc:
        with (
            tc.tile_pool(name="persist", bufs=1) as persist,
            tc.tile_pool(name="pt", bufs=6) as ptp,
            tc.tile_pool(name="small", bufs=2) as small,
            tc.tile_pool(name="ysb", bufs=2) as ysb,
            tc.tile_pool(name="psa", bufs=2, space="PSUM") as psa,
            tc.tile_pool(name="psb", bufs=2, space="PSUM") as psb,
        ):
            # ---- persistent SBUF tiles ----
            def load(dram, rows, cols, tag):
                tiles = []
                for i in range(rows // P):
                    t = persist.tile([P, cols], BF16, tag=f"{tag}{i}",
                                     name=f"{tag}{i}")
                    nc.sync.dma_start(out=t[:, :], in_=dram[i * P:(i + 1) * P, :])
                    tiles.append(t)
                return tiles

            # V with the per-head ones column; memset before the loads so
            # GpSimd works during the DMA head
            vtb = []
            for i in range(KT):
                t = persist.tile([P, H * (DH + 1)], BF16, tag=f"v{i}",
                                 name=f"v{i}")
                nc.gpsimd.memset(t[:, :], 1.0)
                vtb.append(t)

            # E matrix for the denominator broadcast: rb = E.T @ denb
            # E[0, 0:64] = 1 -> den0 lands on out partitions 0..63
            # E[32, 64:128] = 1 -> den1 lands on out partitions 64..127
            emat = persist.tile([33, P], BF16, tag="emat", name="emat")
            nc.gpsimd.memset(emat[:, :], 0.0)
            nc.gpsimd.memset(emat[0:1, 0:DH], 1.0)
            nc.gpsimd.memset(emat[32:33, DH:P], 1.0)
            # denb rows 1..31 stay zero forever; rows 0/32 rewritten per pair
            denb = persist.tile([33, NQ], BF16, tag="denb", name="denb")
            nc.gpsimd.memset(denb[:, :], 0.0)

            # load order: wq + xt unblock QT, then wk, wv, wp, bias
            wqb = load(wqt, C, C, "wqb")
            xtb = load(xt, C, NK, "xtb")
            wkb = load(wkt, C, C, "wkb")
            wvb = load(wvt, C, C, "wvb")
            wpb = load(wpt, C, C, "wpb")
            bpf = persist.tile([P, C], F32, tag="bpf", name="bpf")
            nc.sync.dma_start(out=bpf[:, :], in_=bpb[:, :])

            qtb = [persist.tile([P, NQ], BF16, tag=f"qt{i}", name=f"qt{i}")
                   for i in range(CT)]
            ktb = [persist.tile([P, NK], BF16, tag=f"kt{i}", name=f"kt{i}")
                   for i in range(CT)]
            otb = [persist.tile([P, NQ], BF16, tag=f"ot{i}", name=f"ot{i}")
                   for i in range(CT)]

            # ---- linears, emitted per cout-tile so they interleave ----
            def qt_tile(i):
                ps = psa.tile([P, NQ], F32, tag="a", name="qps")
                for j in range(2):
                    for k in range(CT):
                        nc.tensor.matmul(
                            ps[:, j * QCH:(j + 1) * QCH],
                            lhsT=wqb[k][:, i * P:(i + 1) * P],
                            rhs=xtb[k][:, j * QCH:(j + 1) * QCH],
                            start=(k == 0),
                            stop=(k == CT - 1),
                        )
                nc.vector.tensor_copy(qtb[i][:, :], ps[:, :])

            def kt_tile(i):
                for half in range(2):
                    ps = psa.tile([P, NQ], F32, tag="a", name="kps")
                    for j in range(2):
                        c0 = half * NQ + j * QCH
                        for k in range(CT):
                            nc.tensor.matmul(
                                ps[:, j * QCH:(j + 1) * QCH],
                                lhsT=wkb[k][:, i * P:(i + 1) * P],
                                rhs=xtb[k][:, c0:c0 + QCH],
                                start=(k == 0),
                                stop=(k == CT - 1),
                            )
                    nc.vector.tensor_copy(
                        ktb[i][:, half * NQ:(half + 1) * NQ], ps[:, :])

            def v_chunk(c):
                # V columns c*256..c*256+255 = heads 4c..4c+3 for all k-tiles
                for i in range(KT):
                    ps = psa.tile([P, 4 * DH], F32, tag="a", name="vps")
                    for k in range(CT):
                        nc.tensor.matmul(
                            ps[:, :],
                            lhsT=xtb[k][:, i * P:(i + 1) * P],
                            rhs=wvb[k][:, c * 4 * DH:(c + 1) * 4 * DH],
                            start=(k == 0),
                            stop=(k == CT - 1),
                        )
                    dst = vtb[i][:, :].rearrange(
                        "p (h e) -> p h e", e=DH + 1
                    )[:, 4 * c:4 * c + 4, 0:DH]
                    src = ps[:, :].rearrange("p (h e) -> p h e", e=DH)
                    nc.vector.tensor_copy(dst, src)

            # ---- attention pair ----
            def attention_pair(hp):
                h0, h1 = 2 * hp, 2 * hp + 1
                ot0 = psb.tile([DH + 1, NQ], F32, tag="b", name="ot0")
                ot1 = psb.tile([DH + 1, NQ], F32, tag="b", name="ot1")

                def av_pair(i, pt0, pt1):
                    for j in range(2):
                        nc.tensor.matmul(
                            ot0[:, j * QCH:(j + 1) * QCH],
                            lhsT=vtb[i][:, h0 * (DH + 1):h0 * (DH + 1) + DH + 1],
                            rhs=pt0[:, j * QCH:(j + 1) * QCH],
                            start=(i == 0), stop=(i == KT - 1),
                        )
                        nc.tensor.matmul(
                            ot1[:, j * QCH:(j + 1) * QCH],
                            lhsT=vtb[i][:, h1 * (DH + 1):h1 * (DH + 1) + DH + 1],
                            rhs=pt1[:, j * QCH:(j + 1) * QCH],
                            start=(i == 0), stop=(i == KT - 1),
                        )

                # software pipeline: AV(i-1) is emitted between ST(i) and
                # exp(i), so the PE never stalls on the exp it just fed
                pending = None
                for i in range(KT):
                    st0 = psa.tile([P, NQ], F32, tag="a", name="st0")
                    st1 = psa.tile([P, NQ], F32, tag="a", name="st1")
                    for j in range(2):
                        # heads alternate PE row groups (base 0 / base 64)
                        # -> hardware runs the pair concurrently
                        nc.tensor.matmul(
                            st0[:, j * QCH:(j + 1) * QCH],
                            lhsT=ktb[hp][0:DH, i * P:(i + 1) * P],
                            rhs=qtb[hp][0:DH, j * QCH:(j + 1) * QCH],
                            start=True, stop=True,
                            tile_position=(0, 0),
                        )
                        nc.tensor.matmul(
                            st1[:, j * QCH:(j + 1) * QCH],
                            lhsT=ktb[hp][DH:P, i * P:(i + 1) * P],
                            rhs=qtb[hp][DH:P, j * QCH:(j + 1) * QCH],
                            start=True, stop=True,
                            tile_position=(64, 0),
                        )
                    if pending is not None:
                        av_pair(*pending)
                    pt0 = ptp.tile([P, NQ], BF16, tag="pt", name="pt0")
                    pt1 = ptp.tile([P, NQ], BF16, tag="pt", name="pt1")
                    nc.scalar.activation(
                        pt0[:, :], st0[:, :],
                        mybir.ActivationFunctionType.Exp, scale=SCALE,
                    )
                    nc.scalar.activation(
                        pt1[:, :], st1[:, :],
                        mybir.ActivationFunctionType.Exp, scale=SCALE,
                    )
                    pending = (i, pt0, pt1)
                av_pair(*pending)
                return (hp, ot0, ot1)

            def norm(hp, ot0, ot1):
                # denominators -> bf16 SBUF rows 0 / 32 (32-aligned bases)
                nc.vector.tensor_copy(denb[0:1, :], ot0[DH:DH + 1, :])
                nc.vector.tensor_copy(denb[32:33, :], ot1[DH:DH + 1, :])
                # numerator drains (GpSimd cannot read PSUM, so DVE)
                osb = small.tile([P, NQ], F32, tag="osb", name="osb")
                nc.vector.tensor_copy(osb[0:DH, :], ot0[0:DH, :])
                nc.vector.tensor_copy(osb[DH:P, :], ot1[0:DH, :])
                # PE broadcast of both denominators in one matmul pair
                rb = psa.tile([P, NQ], F32, tag="a", name="rb")
                for j in range(2):
                    nc.tensor.matmul(
                        rb[:, j * QCH:(j + 1) * QCH],
                        lhsT=emat[:, :],
                        rhs=denb[:, j * QCH:(j + 1) * QCH],
                        start=True, stop=True,
                    )
                rc = small.tile([P, NQ], F32, tag="rc", name="rc")
                nc.vector.reciprocal(rc[:, :], rb[:, :])
                nc.vector.tensor_mul(otb[hp][:, :], osb[:, :], rc[:, :])

            # ---- interleaved schedule ----
            qt_tile(0)
            kt_tile(0)
            v_chunk(0)                      # heads 0..3 (pairs 0, 1)
            prev = attention_pair(0)
            qt_tile(1)
            kt_tile(1)
            norm(*prev)
            prev = attention_pair(1)
            v_chunk(1)                      # heads 4..7 (pairs 2, 3)
            qt_tile(2)
            kt_tile(2)
            norm(*prev)
            prev = attention_pair(2)
            qt_tile(3)
            kt_tile(3)
            norm(*prev)
            prev = attention_pair(3)
            v_chunk(2)                      # heads 8..11 (pairs 4, 5)
            qt_tile(4)
            kt_tile(4)
            norm(*prev)
            prev = attention_pair(4)
            qt_tile(5)
            kt_tile(5)
            norm(*prev)
            prev = attention_pair(5)
            norm(*prev)

            # ---- projection: Y[q, co] = OT.T @ WpT + bp ----
            for qi in range(NQ // P):
                yt = ysb.tile([P, C], F32, tag="y", name="yt")
                for (c0, csz) in ((0, QCH), (QCH, C - QCH)):
                    ps = psa.tile([P, csz], F32, tag="a", name="pps")
                    for k in range(CT):
                        nc.tensor.matmul(
                            ps[:, :],
                            lhsT=otb[k][:, qi * P:(qi + 1) * P],
                            rhs=wpb[k][:, c0:c0 + csz],
                            start=(k == 0), stop=(k == CT - 1),
                        )
                    nc.vector.tensor_add(
                        yt[:, c0:c0 + csz], ps[:, :], bpf[:, c0:c0 + csz])
                nc.sync.dma_start(out=y[qi * P:(qi + 1) * P, :], in_=yt[:, :])

    nc.compile()
    return nc


def _get_prog() -> bass.Bass:
    global _PROG
    if _PROG is None:
        _PROG = _build_program()
    return _PROG


def kernel(x, Wq, Wk, Wv, Wp, bp):
    global LAST_RESULT
    import ml_dtypes
    bf16 = ml_dtypes.bfloat16

    x = np.asarray(x, dtype=np.float32)
    wqt = np.ascontiguousarray(np.asarray(Wq, np.float32).T).astype(bf16)
    wkt = np.ascontiguousarray(np.asarray(Wk, np.float32).T).astype(bf16)
    wvt = np.ascontiguousarray(np.asarray(Wv, np.float32).T).astype(bf16)
    wpt = np.ascontiguousarray(np.asarray(Wp, np.float32).T).astype(bf16)
    bpv = np.ascontiguousarray(np.broadcast_to(
        np.asarray(bp, np.float32).reshape(1, C), (P, C)))

    B, N, _ = x.shape
    in_maps = []
    for core in range(8):
        b, qh = core // 2, core % 2
        # roll the key columns so this core's queries are columns 0..1023;
        # K and V see the same permutation, so attention is unchanged
        xtf = x[b].T
        xtr = np.concatenate(
            [xtf[:, qh * NQ:], xtf[:, :qh * NQ]], axis=1)
        in_maps.append({
            "xt": np.ascontiguousarray(xtr).astype(bf16),
            "wqt": wqt, "wkt": wkt, "wvt": wvt, "wpt": wpt, "bpb": bpv,
        })

    res = run_bass_kernel_spmd(
        _get_prog(), in_maps, core_ids=list(range(8)),
        trace=bool(os.environ.get("BASS_TRACE")),
    )
    LAST_RESULT = res

    out = np.empty((B, N, C), np.float32)
    for core in range(8):
        b, qh = core // 2, core % 2
        out[b, qh * NQ:(qh + 1) * NQ, :] = res.results[core]["y"]
    return out


# revision 4
# speedup vs baseline: 1.2120x; 1.1971x over previous
"""Multi-head attention (B=4, N=2048, C=768, H=12) on 8 trn2 NeuronCores.

Sharding: core c handles batch b = c//2 and query rows [ (c%2)*1024, +1024 ).
Each core computes K/V for its full batch (duplicated across the pair),
attention for all 12 heads over its 1024 queries, and the output projection
for its rows. Output gather is pure concatenation (no cross-core reduce).

Host side: all activations/weights are pre-cast to bf16 and x is column-
rotated per core so the core's own query columns are always x.T[:, 0:1024]
(key order is permuted identically in K and V, which leaves attention
invariant). This keeps one SPMD program for all 8 cores.

On-chip layout (per core):
  xt  = x_b.T (rolled)  [768, 2048]  bf16, c on partitions
  QT  = Wq @ xq.T       [768, 1024]  head h rows h*64..h*64+63
  KT  = Wk @ x.T        [768, 2048]
  V   = x @ Wv.T        [2048, 780]  row-tiles of 128 keys; head h in cols
                                     h*65..h*65+63, col h*65+64 == 1.0 (ones
                                     column -> PV matmul also emits softmax
                                     denominators)
  ST_h = (K_h Q_h^T)    [128k, 1024q] PSUM per k-tile; exp on ScalarE with
                                     the 1/8 attention scale folded in
  OT_h = [V_h|1]^T P_h  [65, 1024]   PSUM accumulated over 16 k-tiles;
                                     row 64 = softmax denominators
  norm: denb (bf16) <- den rows; rb = E^T @ denb broadcasts den0 to
        partitions 0..63 and den1 to 64..127 on the PE; reciprocal+multiply
        run full-width on the DVE (no single-partition ops anywhere).
  Y    = OT^T Wp^T + bp [1024, 768]  bias added by the DVE during drain

Program order interleaves the per-pair linears (QT/KT/V chunks) between
attention pairs so the PE has independent work while a pair's denominators
drain, and the Scalar engine (exp) starts as early as possible.
"""

import os
import sys

import numpy as np

sys.path.insert(0, "/opt/trn_rl_repo")

import concourse.bass as bass
from concourse import bacc
import concourse.mybir as mybir
from concourse.tile import TileContext
from concourse.bass_utils import run_bass_kernel_spmd

P = 128
C = 768
NK = 2048
NQ = 1024
H = 12
DH = 64
CT = C // P          # 6 c-tiles (contraction tiles for the linears)
KT = NK // P         # 16 key tiles
QCH = 512            # max psum bank free dim for fp32
SCALE = DH ** -0.5
F32 = mybir.dt.float32
BF16 = mybir.dt.bfloat16

LAST_RESULT = None
_PROG = None


def _build_program() -> bass.Bass:
    nc = bacc.Bacc(None, target_bir_lowering=False)

    xt = nc.dram_tensor("xt", [C, NK], BF16, kind="ExternalInput")
    wqt = nc.dram_tensor("wqt", [C, C], BF16, kind="ExternalInput")
    wkt = nc.dram_tensor("wkt", [C, C], BF16, kind="ExternalInput")
    wvt = nc.dram_tensor("wvt", [C, C], BF16, kind="ExternalInput")
    wpt = nc.dram_tensor("wpt", [C, C], BF16, kind="ExternalInput")
    bpb = nc.dram_tensor("bpb", [P, C], F32, kind="ExternalInput")
    y = nc.dram_tensor("y", [NQ, C], F32, kind="ExternalOutput")

    with TileContext(nc) as tc:
        with (
            tc.tile_pool(name="persist", bufs=1) as persist,
            tc.tile_pool(name="pt", bufs=6) as ptp,
            tc.tile_pool(name="small", bufs=2) as small,
            tc.tile_pool(name="ysb", bufs=2) as ysb,
            tc.tile_pool(name="psa", bufs=2, space="PSUM") as psa,
            tc.tile_pool(name="psb", bufs=2, space="PSUM") as psb,
        ):
            # ---- persistent SBUF tiles ----
            def load(dram, rows, cols, tag):
                tiles = []
                for i in range(rows // P):
                    t = persist.tile([P, cols], BF16, tag=f"{tag}{i}",
                                     name=f"{tag}{i}")
                    nc.sync.dma_start(out=t[:, :], in_=dram[i * P:(i + 1) * P, :])
                    tiles.append(t)
                return tiles

            # V with the per-head ones column; memset before the loads so
            # GpSimd works during the DMA head
            vtb = []
            for i in range(KT):
                t = persist.tile([P, H * (DH + 1)], BF16, tag=f"v{i}",
                                 name=f"v{i}")
                nc.gpsimd.memset(t[:, :], 1.0)
                vtb.append(t)

            # E matrix for the denominator broadcast: rb = E.T @ denb
            # E[0, 0:64] = 1 -> den0 lands on out partitions 0..63
            # E[32, 64:128] = 1 -> den1 lands on out partitions 64..127
            emat = persist.tile([33, P], BF16, tag="emat", name="emat")
            nc.gpsimd.memset(emat[:, :], 0.0)
            nc.gpsimd.memset(emat[0:1, 0:DH], 1.0)
            nc.gpsimd.memset(emat[32:33, DH:P], 1.0)
            # denb rows 1..31 stay zero forever; rows 0/32 rewritten per pair
            denb = persist.tile([33, NQ], BF16, tag="denb", name="denb")
            nc.gpsimd.memset(denb[:, :], 0.0)

            # load order: wq + xt unblock QT, then wk, wv, wp, bias
            wqb = load(wqt, C, C, "wqb")
            xtb = load(xt, C, NK, "xtb")
            wkb = load(wkt, C, C, "wkb")
            wvb = load(wvt, C, C, "wvb")
            wpb = load(wpt, C, C, "wpb")
            bpf = persist.tile([P, C], F32, tag="bpf", name="bpf")
            nc.sync.dma_start(out=bpf[:, :], in_=bpb[:, :])

            qtb = [persist.tile([P, NQ], BF16, tag=f"qt{i}", name=f"qt{i}")
                   for i in range(CT)]
            ktb = [persist.tile([P, NK], BF16, tag=f"kt{i}", name=f"kt{i}")
                   for i in range(CT)]
            otb = [persist.tile([P, NQ], BF16, tag=f"ot{i}", name=f"ot{i}")
                   for i in range(CT)]

            # ---- linears, emitted per cout-tile so they interleave ----
            def qt_tile(i):
                ps = psa.tile([P, NQ], F32, tag="a", name="qps")
                for j in range(2):
                    for k in range(CT):
                        nc.tensor.matmul(
                            ps[:, j * QCH:(j + 1) * QCH],
                            lhsT=wqb[k][:, i * P:(i + 1) * P],
                            rhs=xtb[k][:, j * QCH:(j + 1) * QCH],
                            start=(k == 0),
                            stop=(k == CT - 1),
                        )
                nc.vector.tensor_copy(qtb[i][:, :], ps[:, :])

            def kt_tile(i):
                for half in range(2):
                    ps = psa.tile([P, NQ], F32, tag="a", name="kps")
                    for j in range(2):
                        c0 = half * NQ + j * QCH
                        for k in range(CT):
                            nc.tensor.matmul(
                                ps[:, j * QCH:(j + 1) * QCH],
                                lhsT=wkb[k][:, i * P:(i + 1) * P],
                                rhs=xtb[k][:, c0:c0 + QCH],
                                start=(k == 0),
                                stop=(k == CT - 1),
                            )
                    nc.vector.tensor_copy(
                        ktb[i][:, half * NQ:(half + 1) * NQ], ps[:, :])

            def v_chunk(c):
                # V columns c*256..c*256+255 = heads 4c..4c+3 for all k-tiles
                for i in range(KT):
                    ps = psa.tile([P, 4 * DH], F32, tag="a", name="vps")
                    for k in range(CT):
                        nc.tensor.matmul(
                            ps[:, :],
                            lhsT=xtb[k][:, i * P:(i + 1) * P],
                            rhs=wvb[k][:, c * 4 * DH:(c + 1) * 4 * DH],
                            start=(k == 0),
                            stop=(k == CT - 1),
                        )
                    dst = vtb[i][:, :].rearrange(
                        "p (h e) -> p h e", e=DH + 1
                    )[:, 4 * c:4 * c + 4, 0:DH]
                    src = ps[:, :].rearrange("p (h e) -> p h e", e=DH)
                    nc.vector.tensor_copy(dst, src)

            # ---- attention pair ----
            def attention_pair(hp):
                h0, h1 = 2 * hp, 2 * hp + 1
                ot0 = psb.tile([DH + 1, NQ], F32, tag="b", name="ot0")
                ot1 = psb.tile([DH + 1, NQ], F32, tag="b", name="ot1")

                def av_pair(i, pt0, pt1):
                    for j in range(2):
                        nc.tensor.matmul(
                            ot0[:, j * QCH:(j + 1) * QCH],
                            lhsT=vtb[i][:, h0 * (DH + 1):h0 * (DH + 1) + DH + 1],
                            rhs=pt0[:, j * QCH:(j + 1) * QCH],
                            start=(i == 0), stop=(i == KT - 1),
                        )
                        nc.tensor.matmul(
                            ot1[:, j * QCH:(j + 1) * QCH],
                            lhsT=vtb[i][:, h1 * (DH + 1):h1 * (DH + 1) + DH + 1],
                            rhs=pt1[:, j * QCH:(j + 1) * QCH],
                            start=(i == 0), stop=(i == KT - 1),
                        )

                # software pipeline: AV(i-1) is emitted between ST(i) and
                # exp(i), so the PE never stalls on the exp it just fed
                pending = None
                for i in range(KT):
                    st0 = psa.tile([P, NQ], F32, tag="a", name="st0")
                    st1 = psa.tile([P, NQ], F32, tag="a", name="st1")
                    for j in range(2):
                        # heads alternate PE row groups (base 0 / base 64)
                        # -> hardware runs the pair concurrently
                        nc.tensor.matmul(
                            st0[:, j * QCH:(j + 1) * QCH],
                            lhsT=ktb[hp][0:DH, i * P:(i + 1) * P],
                            rhs=qtb[hp][0:DH, j * QCH:(j + 1) * QCH],
                            start=True, stop=True,
                            tile_position=(0, 0),
                        )
                        nc.tensor.matmul(
                            st1[:, j * QCH:(j + 1) * QCH],
                            lhsT=ktb[hp][DH:P, i * P:(i + 1) * P],
                            rhs=qtb[hp][DH:P, j * QCH:(j + 1) * QCH],
                            start=True, stop=True,
                            tile_position=(64, 0),
                        )
                    if pending is not None:
                        av_pair(*pending)
                    pt0 = ptp.tile([P, NQ], BF16, tag="pt", name="pt0")
                    pt1 = ptp.tile([P, NQ], BF16, tag="pt", name="pt1")
                    nc.scalar.activation(
                        pt0[:, :], st0[:, :],
                        mybir.ActivationFunctionType.Exp, scale=SCALE,
                    )
                    nc.scalar.activation(
                        pt1[:, :], st1[:, :],
                        mybir.ActivationFunctionType.Exp, scale=SCALE,
                    )
                    pending = (i, pt0, pt1)
                av_pair(*pending)
                return (hp, ot0, ot1)

            def norm(hp, ot0, ot1):
                # denominators -> bf16 SBUF rows 0 / 32 (32-aligned bases)
                nc.vector.tensor_copy(denb[0:1, :], ot0[DH:DH + 1, :])
                nc.vector.tensor_copy(denb[32:33, :], ot1[DH:DH + 1, :])
                # numerator drains (GpSimd cannot read PSUM, so DVE)
                osb = small.tile([P, NQ], F32, tag="osb", name="osb")
                nc.vector.tensor_copy(osb[0:DH, :], ot0[0:DH, :])
                nc.vector.tensor_copy(osb[DH:P, :], ot1[0:DH, :])
                # PE broadcast of both denominators in one matmul pair
                rb = psa.tile([P, NQ], F32, tag="a", name="rb")
                for j in range(2):
                    nc.tensor.matmul(
                        rb[:, j * QCH:(j + 1) * QCH],
                        lhsT=emat[:, :],
                        rhs=denb[:, j * QCH:(j + 1) * QCH],
                        start=True, stop=True,
                    )
                rc = small.tile([P, NQ], F32, tag="rc", name="rc")
                nc.vector.reciprocal_approx_fast(rc[:, :], rb[:, :])
                nc.vector.tensor_mul(otb[hp][:, :], osb[:, :], rc[:, :])

            # ---- interleaved schedule ----
            qt_tile(0)
            kt_tile(0)
            v_chunk(0)                      # heads 0..3 (pairs 0, 1)
            prev = attention_pair(0)
            qt_tile(1)
            kt_tile(1)
            norm(*prev)
            prev = attention_pair(1)
            v_chunk(1)                      # heads 4..7 (pairs 2, 3)
            qt_tile(2)
            kt_tile(2)
            norm(*prev)
            prev = attention_pair(2)
            qt_tile(3)
            kt_tile(3)
            norm(*prev)
            prev = attention_pair(3)
            v_chunk(2)                      # heads 8..11 (pairs 4, 5)
            qt_tile(4)
            kt_tile(4)
            norm(*prev)
            prev = attention_pair(4)
            qt_tile(5)
            kt_tile(5)
            norm(*prev)
            prev = attention_pair(5)
            norm(*prev)

            # ---- projection: Y[q, co] = OT.T @ WpT + bp ----
            for qi in range(NQ // P):
                yt = ysb.tile([P, C], F32, tag="y", name="yt")
                for (c0, csz) in ((0, QCH), (QCH, C - QCH)):
                    ps = psa.tile([P, csz], F32, tag="a", name="pps")
                    for k in range(CT):
                        nc.tensor.matmul(
                            ps[:, :],
                            lhsT=otb[k][:, qi * P:(qi + 1) * P],
                            rhs=wpb[k][:, c0:c0 + csz],
                            start=(k == 0), stop=(k == CT - 1),
                        )
                    nc.vector.tensor_add(
                        yt[:, c0:c0 + csz], ps[:, :], bpf[:, c0:c0 + csz])
                nc.sync.dma_start(out=y[qi * P:(qi + 1) * P, :], in_=yt[:, :])

    nc.compile()
    return nc


def _get_prog() -> bass.Bass:
    global _PROG
    if _PROG is None:
        _PROG = _build_program()
    return _PROG


def kernel(x, Wq, Wk, Wv, Wp, bp):
    global LAST_RESULT
    import ml_dtypes
    bf16 = ml_dtypes.bfloat16

    x = np.asarray(x, dtype=np.float32)
    wqt = np.ascontiguousarray(np.asarray(Wq, np.float32).T).astype(bf16)
    wkt = np.ascontiguousarray(np.asarray(Wk, np.float32).T).astype(bf16)
    wvt = np.ascontiguousarray(np.asarray(Wv, np.float32).T).astype(bf16)
    wpt = np.ascontiguousarray(np.asarray(Wp, np.float32).T).astype(bf16)
    bpv = np.ascontiguousarray(np.broadcast_to(
        np.asarray(bp, np.float32).reshape(1, C), (P, C)))

    B, N, _ = x.shape
    in_maps = []
    for core in range(8):
        b, qh = core // 2, core % 2
        # roll the key columns so this core's queries are columns 0..1023;
        # K and V see the same permutation, so attention is unchanged
        xtf = x[b].T
        xtr = np.concatenate(
            [xtf[:, qh * NQ:], xtf[:, :qh * NQ]], axis=1)
        in_maps.append({
            "xt": np.ascontiguousarray(xtr).astype(bf16),
            "wqt": wqt, "wkt": wkt, "wvt": wvt, "wpt": wpt, "bpb": bpv,
        })

    res = run_bass_kernel_spmd(
        _get_prog(), in_maps, core_ids=list(range(8)),
        trace=bool(os.environ.get("BASS_TRACE")),
    )
    LAST_RESULT = res

    out = np.empty((B, N, C), np.float32)
    for core in range(8):
        b, qh = core // 2, core % 2
        out[b, qh * NQ:(qh + 1) * NQ, :] = res.results[core]["y"]
    return out
